# revision 12
# baseline (speedup 1.0000x reference)
"""AttnDecoderRNN single-step decoder, tensor-parallel over 8 TRN2 NeuronCores.

Sharding (hardcoded for H=1024, V=50257, L=50, ENC=2048, 8 cores):
  - attn_w_W col-sharded fp32; attn_v_W row-sharded; comb_W col-sharded bf16
  - gru_wih / gru_whh row-sharded bf16 (K-sharded partials, AllReduced)
  - out_W col-sharded bf16 over vocab, padded 50257 -> 8*6400 = 51200,
    streamed as PE-stationary [128h,128v] tiles accumulating z^T [128,50]
  - embedding: host selects the one needed row (extreme vocab shard)
  - log_softmax via sharded sum-exp + scalar AllReduce (no max subtraction:
    logits for this model are O(3), exp is safe in fp32)
Collectives in stream order: e[50] AR, gh[3072] AR (attention-independent,
hidden behind the entry-barrier/e-AR), gi[3072] AR, sumexp[1] AR.
"""

import os
import numpy as np
import ml_dtypes

H = 1024
V = 50257
L = 50
ENC = 2048
NCORES = 8
KT = H // 128               # 8 h-chunks of 128
NVC = 50                    # vocab chunks of 128 per core
VS = NVC * 128              # 6400 per-core padded vocab shard (8*6400=51200)
VPAD = NCORES * VS
VCBLK = [4] * 12 + [2]          # vocab-chunk blocks for the ow stream
PAD_BIAS = -1.0e9           # pad-lane bias so exp() -> 0

_CACHE = {}


def _build():
    import concourse.bass as bass
    import concourse.mybir as mybir
    import concourse.tile as tile
    from concourse import bacc

    f32 = mybir.dt.float32
    bf16 = mybir.dt.bfloat16
    AF = mybir.ActivationFunctionType
    nc = bacc.Bacc("TRN2", target_bir_lowering=False, debug=False,
                   num_devices=NCORES)

    def din(name, shape, dt=f32):
        return nc.declare_dram_parameter(name, list(shape), dt, isOutput=False)

    def dout(name, shape):
        return nc.declare_dram_parameter(name, list(shape), f32, isOutput=True)

    # inputs (per-core shards, host-prearranged layouts)
    h_t = din("h_full", [1, H])
    enc_t = din("enc", [L, ENC])
    encT_t = din("encT", [128, ENC // 128, L])
    aw_t = din("aw", [128, 24, 128])
    cw_t = din("cw", [128, 24, 128])
    # merged small columns: [hT(8) | embT(8) | ab | av | cb | hc] = [128, 20]
    sm_t = din("smalls", [128, 2 * KT + 4])
    wih_t = din("wih", [128, 3 * H])
    whh_t = din("whh", [128, 3 * H])
    brz_t = din("b_rz", [1, 2 * H])
    bin_t = din("b_in", [1, H])
    bhn_t = din("b_hn", [1, H])
    ow_t = din("ow", [128, KT * VS], bf16)   # [p, (vc, k, c)] tile-major
    obT_t = din("obT", [128, NVC])           # [p, vc] = ob[vc*128+p]
    # outputs
    zT_t = dout("zT_out", [128, NVC])        # [p, vc] = logprob[vc*128+p]
    hn_t = dout("hn_out", [1, H])
    atw_t = dout("attw_out", [1, L])
    # collective bounce buffers (internal DRAM)
    e_in = nc.dram_tensor("cc_e_in", [1, L], f32)
    e_out = nc.dram_tensor("cc_e_out", [1, L], f32)
    gh_in = nc.dram_tensor("cc_gh_in", [1, 3 * H], f32)
    gh_out = nc.dram_tensor("cc_gh_out", [1, 3 * H], f32)
    gi_in = nc.dram_tensor("cc_gi_in", [1, 3 * H], f32)
    gi_out = nc.dram_tensor("cc_gi_out", [1, 3 * H], f32)
    s_in = nc.dram_tensor("cc_s_in", [1, 1], f32)
    s_out = nc.dram_tensor("cc_s_out", [1, 1], f32)

    RG = [list(range(NCORES))]

    def allreduce(src, dst):
        nc.gpsimd.collective_compute(
            "AllReduce", mybir.AluOpType.add, replica_groups=RG,
            ins=[src[:].opt()], outs=[dst[:].opt()])

    with tile.TileContext(nc) as tc:
        with (
            tc.tile_pool(name="wpool", bufs=1) as wp,
            tc.tile_pool(name="bigw", bufs=2) as bw,
            tc.tile_pool(name="spool", bufs=1) as sp,
            tc.tile_pool(name="owpool", bufs=6) as owp,
            tc.tile_pool(name="psA", bufs=3, space="PSUM") as psA,
            tc.tile_pool(name="psZ", bufs=2, space="PSUM") as psZ,
            tc.tile_pool(name="psT", bufs=1, space="PSUM") as psT,
        ):
            dma = nc.sync.dma_start

            # ---- persistent loads, critical-chain first -----------------
            sm_sb = wp.tile([128, 2 * KT + 4], f32)
            dma(out=sm_sb, in_=sm_t[:])
            hT_sb = sm_sb[:, 0:KT]
            embT_sb = sm_sb[:, KT:2 * KT]
            ab_sb = sm_sb[:, 2 * KT:2 * KT + 1]
            av_sb = sm_sb[:, 2 * KT + 1:2 * KT + 2]
            cb_sb = sm_sb[:, 2 * KT + 2:2 * KT + 3]
            hc_sb = sm_sb[:, 2 * KT + 3:2 * KT + 4]
            encT_sb = wp.tile([128, ENC // 128, L], f32)
            dma(out=encT_sb, in_=encT_t[:])
            enc_sb = wp.tile([L, ENC], f32)
            dma(out=enc_sb, in_=enc_t[:])
            aw_sb = bw.tile([128, 24, 128], f32, tag="bigw")
            dma(out=aw_sb, in_=aw_t[:])
            whh_sb = bw.tile([128, 3 * H], f32, tag="bigw")
            nc.scalar.dma_start(out=whh_sb, in_=whh_t[:])
            cw_sb = bw.tile([128, 24, 128], f32, tag="bigw")
            nc.scalar.dma_start(out=cw_sb, in_=cw_t[:])
            wih_sb = bw.tile([128, 3 * H], f32, tag="bigw")
            nc.scalar.dma_start(out=wih_sb, in_=wih_t[:])
            brz_sb = wp.tile([1, 2 * H], f32)
            nc.scalar.dma_start(out=brz_sb, in_=brz_t[:])
            bin_sb = wp.tile([1, H], f32)
            nc.scalar.dma_start(out=bin_sb, in_=bin_t[:])
            bhn_sb = wp.tile([1, H], f32)
            nc.scalar.dma_start(out=bhn_sb, in_=bhn_t[:])
            hrow_sb = wp.tile([1, H], f32)
            nc.scalar.dma_start(out=hrow_sb, in_=h_t[:])
            obT_sb = wp.tile([128, NVC], f32)
            nc.scalar.dma_start(out=obT_sb, in_=obT_t[:])

            ones_sb = wp.tile([1, 1], f32)
            nc.vector.memset(ones_sb, 1.0)
            ones128 = wp.tile([128, 1], f32)
            nc.vector.memset(ones128, 1.0)

            # ---- attention: e_partial = relu(att_cat @ aw + ab) @ av ----
            v1_ps = psA.tile([128, 1], f32, tag="a")
            for k in range(KT):
                nc.tensor.matmul(v1_ps, aw_sb[:, k, :], hT_sb[:, k:k + 1],
                                 start=(k == 0), stop=(k == KT - 1))
            v1_sb = sp.tile([128, 1], f32)
            nc.vector.tensor_copy(v1_sb, v1_ps)
            bias2 = sp.tile([128, 1], f32)
            nc.vector.tensor_add(bias2, v1_sb, ab_sb)
            ep = psA.tile([128, L], f32, tag="a")
            for k in range(16):
                nc.tensor.matmul(ep, aw_sb[:, KT + k, :], encT_sb[:, k, :],
                                 start=(k == 0), stop=(k == 15))
            r_sb = sp.tile([128, L], f32)
            nc.scalar.activation(r_sb, ep, AF.Relu, bias=bias2)
            e_ps = psA.tile([1, L], f32, tag="a")
            nc.tensor.matmul(e_ps, av_sb, r_sb, start=True, stop=True)
            e_sb = sp.tile([1, L], f32)
            nc.vector.tensor_copy(e_sb, e_ps)
            dma(out=e_in[:], in_=e_sb)
            allreduce(e_in, e_out)            # CC stream slot 1

            # gh partials (attention-independent) ride behind the e-AR
            for j in range(6):
                zp = psZ.tile([1, 512], f32, tag="z")
                nc.tensor.matmul(zp, hc_sb, whh_sb[:, j * 512:(j + 1) * 512],
                                 start=True, stop=True)
                gcp = sp.tile([1, 512], f32, tag="gcp")
                if j % 2 == 0:
                    nc.vector.tensor_copy(gcp, zp)
                else:
                    nc.scalar.copy(gcp, zp)
                dma(out=gh_in[0:1, j * 512:(j + 1) * 512], in_=gcp)
            allreduce(gh_in, gh_out)          # CC stream slot 2

            # x^T emb-part (attention-independent), fills the e-AR wait
            x_ps = psA.tile([128, 1], f32, tag="a")
            for k in range(KT):
                nc.tensor.matmul(x_ps, cw_sb[:, k, :], embT_sb[:, k:k + 1],
                                 start=(k == 0), stop=False)

            e_row = sp.tile([1, L], f32)
            dma(out=e_row, in_=e_out[:])

            # softmax over L (with max subtraction; all on partition 0)
            m1 = sp.tile([1, 1], f32)
            nc.vector.reduce_max(m1, e_row, axis=mybir.AxisListType.X)
            negm = sp.tile([1, 1], f32)
            nc.scalar.mul(negm, m1, -1.0)
            ew = sp.tile([1, L], f32)
            s_att = sp.tile([1, 1], f32)
            nc.scalar.activation(ew, e_row, AF.Exp, bias=negm,
                                 accum_out=s_att)
            inv = sp.tile([1, 1], f32)
            nc.vector.reciprocal(inv, s_att)
            attw_sb = sp.tile([1, L], f32)
            nc.scalar.mul(attw_sb, ew, inv)
            dma(out=atw_t[:], in_=attw_sb)

            # attn_w^T [L,1] via K=1 matmul with ones
            awT_ps = psA.tile([L, 1], f32, tag="a")
            nc.tensor.matmul(awT_ps, attw_sb, ones_sb, start=True, stop=True)
            awT_sb = sp.tile([L, 1], f32)
            nc.vector.tensor_copy(awT_sb, awT_ps)

            # attn_applied^T chunks: aaT[:, j] = enc[:, j*128:+128]^T @ attn_w^T
            aa_ps = psA.tile([128, ENC // 128], f32, tag="a")
            for j in range(ENC // 128):
                nc.tensor.matmul(aa_ps[:, j:j + 1],
                                 enc_sb[:, j * 128:(j + 1) * 128], awT_sb,
                                 start=True, stop=True)
            aaT_sb = sp.tile([128, ENC // 128], f32)
            nc.vector.tensor_copy(aaT_sb, aa_ps)

            # x^T attention part completes the accumulation
            for k in range(KT, 24):
                nc.tensor.matmul(x_ps, cw_sb[:, k, :],
                                 aaT_sb[:, k - KT:k - KT + 1],
                                 start=False, stop=(k == 23))
            xT_sb = sp.tile([128, 1], f32)
            nc.scalar.activation(xT_sb, x_ps, AF.Relu, bias=cb_sb)

            # gi partials -> AllReduce
            for j in range(6):
                zp = psZ.tile([1, 512], f32, tag="z")
                nc.tensor.matmul(zp, xT_sb, wih_sb[:, j * 512:(j + 1) * 512],
                                 start=True, stop=True)
                gcp = sp.tile([1, 512], f32, tag="gcp")
                if j % 2 == 0:
                    nc.vector.tensor_copy(gcp, zp)
                else:
                    nc.scalar.copy(gcp, zp)
                dma(out=gi_in[0:1, j * 512:(j + 1) * 512], in_=gcp)
            allreduce(gi_in, gi_out)          # CC stream slot 3

            ggi = sp.tile([1, 3 * H], f32)
            dma(out=ggi, in_=gi_out[:])
            ggh = sp.tile([1, 3 * H], f32)
            dma(out=ggh, in_=gh_out[:])

            # gates (PyTorch order r,z,n), all on partition 0
            t1 = sp.tile([1, 2 * H], f32)
            nc.vector.tensor_add(t1, ggi[:, 0:2 * H], ggh[:, 0:2 * H])
            nc.vector.tensor_add(t1, t1, brz_sb)
            rz = sp.tile([1, 2 * H], f32)
            nc.scalar.activation(rz, t1, AF.Sigmoid)
            hnt = sp.tile([1, H], f32)
            nc.vector.tensor_add(hnt, ggh[:, 2 * H:3 * H], bhn_sb)
            npre = sp.tile([1, H], f32)
            nc.vector.tensor_add(npre, ggi[:, 2 * H:3 * H], bin_sb)
            nc.vector.tensor_mul(hnt, rz[:, 0:H], hnt)
            nc.vector.tensor_add(npre, npre, hnt)
            nn = sp.tile([1, H], f32)
            nc.scalar.activation(nn, npre, AF.Tanh)
            dd = sp.tile([1, H], f32)
            nc.vector.tensor_sub(dd, hrow_sb, nn)
            nc.vector.tensor_mul(dd, rz[:, H:2 * H], dd)
            hnew = sp.tile([1, H], f32)
            nc.vector.tensor_add(hnew, nn, dd)
            dma(out=hn_t[:], in_=hnew)

            # h_new^T chunks [128, KT] (bf16) via K=1 matmuls with ones
            hnT_ps = psA.tile([128, KT], f32, tag="a")
            for k in range(KT):
                nc.tensor.matmul(hnT_ps[:, k:k + 1],
                                 hnew[:, k * 128:(k + 1) * 128], ones_sb,
                                 start=True, stop=True)
            hnT_sb = sp.tile([128, KT], bf16)
            nc.vector.tensor_copy(hnT_sb, hnT_ps)

            # ---- streamed output projection: z^T[p, vc] accumulation ----
            zT_ps = psT.tile([128, NVC], f32, tag="zt")
            vc0 = 0
            for nvc in VCBLK:
                owt = owp.tile([128, nvc * KT * 128], bf16, tag="ow")
                off = KT * 128 * vc0
                nc.scalar.dma_start(
                    out=owt, in_=ow_t[:, off:off + nvc * KT * 128])
                for v in range(nvc):
                    for k in range(KT):
                        nc.tensor.matmul(
                            zT_ps[:, vc0 + v:vc0 + v + 1],
                            owt[:, (v * KT + k) * 128:(v * KT + k + 1) * 128],
                            hnT_sb[:, k:k + 1],
                            start=(k == 0), stop=(k == KT - 1))
                vc0 += nvc

            # epilogue: z = z^T + obT; sumexp; AllReduce; out = z - ln(S)
            zb = sp.tile([128, NVC], f32)
            nc.vector.tensor_add(zb, zT_ps, obT_sb)
            ez = sp.tile([128, NVC], f32)
            acc = sp.tile([128, 1], f32)
            nc.scalar.activation(ez, zb, AF.Exp, accum_out=acc)
            se_ps = psA.tile([1, 1], f32, tag="a")
            nc.tensor.matmul(se_ps, acc, ones128, start=True, stop=True)
            s_loc = sp.tile([1, 1], f32)
            nc.vector.tensor_copy(s_loc, se_ps)
            dma(out=s_in[:], in_=s_loc)
            allreduce(s_in, s_out)            # CC stream slot 4
            sb128 = sp.tile([128, 1], f32)
            bcast = bass.AP(tensor=s_out, offset=0, ap=[[0, 128], [1, 1]])
            dma(out=sb128, in_=bcast)
            lnv = sp.tile([128, 1], f32)
            nc.scalar.activation(lnv, sb128, AF.Ln)
            nlse = sp.tile([128, 1], f32)
            nc.scalar.mul(nlse, lnv, -1.0)
            outT = sp.tile([128, NVC], f32)
            nc.scalar.activation(outT, zb, AF.Identity, bias=nlse)
            dma(out=zT_t[:], in_=outT)

    nc.compile()
    return nc


def shard_inputs(inputs):
    """Full inputs -> list of 8 per-core in_maps."""
    f = lambda x: np.ascontiguousarray(np.asarray(x), dtype=np.float32)
    idx = int(np.asarray(inputs["input_idx"]).reshape(-1)[0])
    h = f(inputs["hidden"]).reshape(1, H)
    enc = f(inputs["encoder_outputs"])                      # [L, ENC]
    emb_row = f(inputs["emb"][idx]).reshape(1, H)
    aw = f(inputs["attn_w_W"])                              # [3H, H]
    ab = f(inputs["attn_w_b"])                              # [H]
    av = f(inputs["attn_v_W"])                              # [H, 1]
    cw = f(inputs["comb_W"])
    cb = f(inputs["comb_b"])
    wih = f(inputs["gru_wih"])                              # [H, 3H]
    whh = f(inputs["gru_whh"])
    bih = f(inputs["gru_bih"])                              # [3H]
    bhh = f(inputs["gru_bhh"])
    ow = f(inputs["out_W"])                                 # [H, V]
    ob = f(inputs["out_b"])                                 # [V]

    ow_pad = np.full((H, VPAD), 0.0, dtype=np.float32)
    ow_pad[:, :V] = ow
    ob_pad = np.full((VPAD,), PAD_BIAS, dtype=np.float32)
    ob_pad[:V] = ob

    b_rz = (bih[:2 * H] + bhh[:2 * H]).reshape(1, 2 * H)
    b_in = bih[2 * H:].reshape(1, H)
    b_hn = bhh[2 * H:].reshape(1, H)
    # [128, 16, 50]: [p, t, l] = enc[l, t*128+p]
    encTr = np.ascontiguousarray(enc.T.reshape(ENC // 128, 128, L)
                                 .transpose(1, 0, 2))
    hT = np.ascontiguousarray(h.reshape(KT, 128).T)          # [128, KT]
    embT = np.ascontiguousarray(emb_row.reshape(KT, 128).T)  # [128, KT]

    def colshard_kpm(w, s, dt=np.float32):
        # [3H, 128] col shard -> [128, 24, 128]: [p, t, m] = w[t*128+p, s][m]
        return np.ascontiguousarray(
            w[:, s].reshape(24, 128, 128).transpose(1, 0, 2).astype(dt))

    bf = ml_dtypes.bfloat16
    maps = []
    for c in range(NCORES):
        s = slice(c * 128, (c + 1) * 128)
        vs = slice(c * VS, (c + 1) * VS)
        # ow tiles: [p, (vc, k, c)] with [128h,128v] PE-stationary tiles
        ow4 = ow_pad[:, vs].reshape(KT, 128, NVC, 128)   # [k, p, vc, c]
        owr = np.ascontiguousarray(
            ow4.transpose(1, 2, 0, 3).reshape(128, KT * VS).astype(bf))
        obT = np.ascontiguousarray(
            ob_pad[vs].reshape(NVC, 128).T)              # [128, NVC]
        sm = np.concatenate([
            hT, embT,
            ab[s].reshape(128, 1), av[s, :].reshape(128, 1),
            cb[s].reshape(128, 1), h[0, s].reshape(128, 1),
        ], axis=1).astype(np.float32)
        maps.append({
            "h_full": h,
            "enc": enc,
            "encT": encTr,
            "aw": colshard_kpm(aw, s),
            "cw": colshard_kpm(cw, s),
            "smalls": np.ascontiguousarray(sm),
            "wih": np.ascontiguousarray(wih[s, :]),
            "whh": np.ascontiguousarray(whh[s, :]),
            "b_rz": b_rz,
            "b_in": b_in,
            "b_hn": b_hn,
            "ow": owr,
            "obT": obT,
        })
    return maps


def assemble_outputs(results):
    # zT [128, NVC] per core: logprob[vc*128+p] = zT[p, vc]
    shards = [np.asarray(results[c]["zT_out"]).T.reshape(-1)
              for c in range(NCORES)]
    z = np.concatenate(shards)[:V].reshape(1, V)
    hn = np.asarray(results[0]["hn_out"]).reshape(1, 1, H).astype(np.float32)
    aw = np.asarray(results[0]["attw_out"]).astype(np.float32)
    return np.ascontiguousarray(z, dtype=np.float32), hn, aw


def kernel(**inputs):
    from concourse import bass_utils
    if "nc" not in _CACHE:
        _CACHE["nc"] = _build()
    nc = _CACHE["nc"]
    in_maps = shard_inputs(inputs)
    trace = bool(int(os.environ.get("KERNEL_TRACE", "0")))
    res = bass_utils.run_bass_kernel_spmd(
        nc, in_maps, core_ids=list(range(NCORES)), trace=trace)
    _CACHE["last_result"] = res
    return assemble_outputs(res.results)


# revision 14
# speedup vs baseline: 1.0135x; 1.0135x over previous
"""AttnDecoderRNN single-step decoder, tensor-parallel over 8 TRN2 NeuronCores.

Sharding (hardcoded for H=1024, V=50257, L=50, ENC=2048, 8 cores):
  - attn_w_W col-sharded fp32; attn_v_W row-sharded; comb_W col-sharded bf16
  - gru_wih / gru_whh row-sharded bf16 (K-sharded partials, AllReduced)
  - out_W col-sharded bf16 over vocab, padded 50257 -> 8*6400 = 51200,
    streamed as PE-stationary [128h,128v] tiles accumulating z^T [128,50]
  - embedding: host selects the one needed row (extreme vocab shard)
  - log_softmax via sharded sum-exp + scalar AllReduce (no max subtraction:
    logits for this model are O(3), exp is safe in fp32)
Collectives in stream order: e[50] AR, gh[3072] AR (attention-independent,
hidden behind the entry-barrier/e-AR), gi[3072] AR, sumexp[1] AR.
"""

import os
import numpy as np
import ml_dtypes

H = 1024
V = 50257
L = 50
ENC = 2048
NCORES = 8
KT = H // 128               # 8 h-chunks of 128
NVC = 50                    # vocab chunks of 128 per core
VS = NVC * 128              # 6400 per-core padded vocab shard (8*6400=51200)
VPAD = NCORES * VS
VCBLK = [4] * 12 + [2]          # vocab-chunk blocks for the ow stream
PAD_BIAS = -1.0e9           # pad-lane bias so exp() -> 0

_CACHE = {}


def _build():
    import concourse.bass as bass
    import concourse.mybir as mybir
    import concourse.tile as tile
    from concourse import bacc

    f32 = mybir.dt.float32
    bf16 = mybir.dt.bfloat16
    AF = mybir.ActivationFunctionType
    nc = bacc.Bacc("TRN2", target_bir_lowering=False, debug=False,
                   num_devices=NCORES)

    def din(name, shape, dt=f32):
        return nc.declare_dram_parameter(name, list(shape), dt, isOutput=False)

    def dout(name, shape):
        return nc.declare_dram_parameter(name, list(shape), f32, isOutput=True)

    # inputs (per-core shards, host-prearranged layouts)
    h_t = din("h_full", [1, H])
    enc_t = din("enc", [L, ENC])
    encT_t = din("encT", [128, ENC // 128, L])
    aw_t = din("aw", [128, 24, 128])
    cw_t = din("cw", [128, 24, 128])
    # merged small columns: [hT(8) | embT(8) | ab | av | cb | hc] = [128, 20]
    sm_t = din("smalls", [128, 2 * KT + 4])
    wih_t = din("wih", [128, 3 * H])
    whh_t = din("whh", [128, 3 * H])
    bih8_t = din("bih8", [1, 3 * H])
    bhh8_t = din("bhh8", [1, 3 * H])
    ow_t = din("ow", [128, KT * VS], bf16)   # [p, (vc, k, c)] tile-major
    obT_t = din("obT", [128, NVC])           # [p, vc] = ob[vc*128+p]
    # outputs
    zT_t = dout("zT_out", [128, NVC])        # [p, vc] = logprob[vc*128+p]
    hn_t = dout("hn_out", [1, H])
    atw_t = dout("attw_out", [1, L])
    # collective bounce buffers (internal DRAM)
    d_in = nc.dram_tensor("cc_d_in", [1, 1], f32)
    d_out = nc.dram_tensor("cc_d_out", [1, 1], f32)
    e_in = nc.dram_tensor("cc_e_in", [1, L], f32)
    e_out = nc.dram_tensor("cc_e_out", [1, L], f32)
    gh_in = nc.dram_tensor("cc_gh_in", [1, 3 * H], f32)
    gh_out = nc.dram_tensor("cc_gh_out", [1, 3 * H], f32)
    gi_in = nc.dram_tensor("cc_gi_in", [1, 3 * H], f32)
    gi_out = nc.dram_tensor("cc_gi_out", [1, 3 * H], f32)
    s_in = nc.dram_tensor("cc_s_in", [1, 1], f32)
    s_out = nc.dram_tensor("cc_s_out", [1, 1], f32)

    RG = [list(range(NCORES))]

    def allreduce(src, dst):
        nc.gpsimd.collective_compute(
            "AllReduce", mybir.AluOpType.add, replica_groups=RG,
            ins=[src[:].opt()], outs=[dst[:].opt()])

    with tile.TileContext(nc) as tc:
        with (
            tc.tile_pool(name="wpool", bufs=1) as wp,
            tc.tile_pool(name="bigw", bufs=2) as bw,
            tc.tile_pool(name="spool", bufs=1) as sp,
            tc.tile_pool(name="owpool", bufs=6) as owp,
            tc.tile_pool(name="psA", bufs=3, space="PSUM") as psA,
            tc.tile_pool(name="psZ", bufs=2, space="PSUM") as psZ,
            tc.tile_pool(name="psT", bufs=1, space="PSUM") as psT,
        ):
            dma = nc.sync.dma_start

            # ---- persistent loads, critical-chain first -----------------
            sm_sb = wp.tile([128, 2 * KT + 4], f32)
            dma(out=sm_sb, in_=sm_t[:])
            hT_sb = sm_sb[:, 0:KT]
            embT_sb = sm_sb[:, KT:2 * KT]
            ab_sb = sm_sb[:, 2 * KT:2 * KT + 1]
            av_sb = sm_sb[:, 2 * KT + 1:2 * KT + 2]
            cb_sb = sm_sb[:, 2 * KT + 2:2 * KT + 3]
            hc_sb = sm_sb[:, 2 * KT + 3:2 * KT + 4]
            encT_sb = wp.tile([128, ENC // 128, L], f32)
            dma(out=encT_sb, in_=encT_t[:])
            enc_sb = wp.tile([L, ENC], f32)
            dma(out=enc_sb, in_=enc_t[:])
            aw_sb = bw.tile([128, 24, 128], f32, tag="bigw")
            dma(out=aw_sb, in_=aw_t[:])
            whh_sb = bw.tile([128, 3 * H], f32, tag="bigw")
            nc.scalar.dma_start(out=whh_sb, in_=whh_t[:])
            cw_sb = bw.tile([128, 24, 128], f32, tag="bigw")
            nc.scalar.dma_start(out=cw_sb, in_=cw_t[:])
            wih_sb = bw.tile([128, 3 * H], f32, tag="bigw")
            nc.scalar.dma_start(out=wih_sb, in_=wih_t[:])
            bih8_sb = wp.tile([1, 3 * H], f32)
            nc.scalar.dma_start(out=bih8_sb, in_=bih8_t[:])
            bhh8_sb = wp.tile([1, 3 * H], f32)
            nc.scalar.dma_start(out=bhh8_sb, in_=bhh8_t[:])
            hrow_sb = wp.tile([1, H], f32)
            nc.scalar.dma_start(out=hrow_sb, in_=h_t[:])
            obT_sb = wp.tile([128, NVC], f32)
            nc.scalar.dma_start(out=obT_sb, in_=obT_t[:])

            ones_sb = wp.tile([1, 1], f32)
            nc.vector.memset(ones_sb, 1.0)
            ones128 = wp.tile([128, 1], f32)
            nc.vector.memset(ones128, 1.0)

            # warm-up collective: absorbs the entry barrier + first-CC cost
            dz = wp.tile([1, 1], f32)
            nc.vector.memset(dz, 0.0)
            dma(out=d_in[:], in_=dz)
            allreduce(d_in, d_out)            # CC stream slot 0
            dwb = sp.tile([1, 1], f32)
            dma(out=dwb, in_=d_out[:])

            # ---- attention: e_partial = relu(att_cat @ aw + ab) @ av ----
            v1_ps = psA.tile([128, 1], f32, tag="a")
            for k in range(KT):
                nc.tensor.matmul(v1_ps, aw_sb[:, k, :], hT_sb[:, k:k + 1],
                                 start=(k == 0), stop=(k == KT - 1))
            v1_sb = sp.tile([128, 1], f32)
            nc.vector.tensor_copy(v1_sb, v1_ps)
            bias2 = sp.tile([128, 1], f32)
            nc.vector.tensor_add(bias2, v1_sb, ab_sb)
            ep = psA.tile([128, L], f32, tag="a")
            for k in range(16):
                nc.tensor.matmul(ep, aw_sb[:, KT + k, :], encT_sb[:, k, :],
                                 start=(k == 0), stop=(k == 15))
            r_sb = sp.tile([128, L], f32)
            nc.scalar.activation(r_sb, ep, AF.Relu, bias=bias2)
            e_ps = psA.tile([1, L], f32, tag="a")
            nc.tensor.matmul(e_ps, av_sb, r_sb, start=True, stop=True)
            e_sb = sp.tile([1, L], f32)
            nc.vector.tensor_copy(e_sb, e_ps)
            dma(out=e_in[:], in_=e_sb)
            allreduce(e_in, e_out)            # CC stream slot 1

            # gh partials (attention-independent) ride behind the e-AR
            for j in range(6):
                zp = psZ.tile([1, 512], f32, tag="z")
                nc.tensor.matmul(zp, hc_sb, whh_sb[:, j * 512:(j + 1) * 512],
                                 start=True, stop=True)
                gcp = sp.tile([1, 512], f32, tag="gcp")
                nc.vector.tensor_add(gcp, zp,
                                     bhh8_sb[:, j * 512:(j + 1) * 512])
                dma(out=gh_in[0:1, j * 512:(j + 1) * 512], in_=gcp)
            allreduce(gh_in, gh_out)          # CC stream slot 2

            # x^T emb-part (attention-independent), fills the e-AR wait
            x_ps = psA.tile([128, 1], f32, tag="a")
            for k in range(KT):
                nc.tensor.matmul(x_ps, cw_sb[:, k, :], embT_sb[:, k:k + 1],
                                 start=(k == 0), stop=False)

            e_row = sp.tile([1, L], f32)
            dma(out=e_row, in_=e_out[:])

            # softmax over L (with max subtraction; all on partition 0)
            m1 = sp.tile([1, 1], f32)
            nc.vector.reduce_max(m1, e_row, axis=mybir.AxisListType.X)
            negm = sp.tile([1, 1], f32)
            nc.scalar.mul(negm, m1, -1.0)
            ew = sp.tile([1, L], f32)
            s_att = sp.tile([1, 1], f32)
            nc.scalar.activation(ew, e_row, AF.Exp, bias=negm,
                                 accum_out=s_att)
            inv = sp.tile([1, 1], f32)
            nc.vector.reciprocal(inv, s_att)
            attw_sb = sp.tile([1, L], f32)
            nc.scalar.mul(attw_sb, ew, inv)
            dma(out=atw_t[:], in_=attw_sb)

            # attn_w^T [L,1] via K=1 matmul with ones
            awT_ps = psA.tile([L, 1], f32, tag="a")
            nc.tensor.matmul(awT_ps, attw_sb, ones_sb, start=True, stop=True)
            awT_sb = sp.tile([L, 1], f32)
            nc.vector.tensor_copy(awT_sb, awT_ps)

            # attn_applied^T chunks: aaT[:, j] = enc[:, j*128:+128]^T @ attn_w^T
            aa_ps = psA.tile([128, ENC // 128], f32, tag="a")
            for j in range(ENC // 128):
                nc.tensor.matmul(aa_ps[:, j:j + 1],
                                 enc_sb[:, j * 128:(j + 1) * 128], awT_sb,
                                 start=True, stop=True)
            aaT_sb = sp.tile([128, ENC // 128], f32)
            nc.vector.tensor_copy(aaT_sb, aa_ps)

            # x^T attention part completes the accumulation
            for k in range(KT, 24):
                nc.tensor.matmul(x_ps, cw_sb[:, k, :],
                                 aaT_sb[:, k - KT:k - KT + 1],
                                 start=False, stop=(k == 23))
            xT_sb = sp.tile([128, 1], f32)
            nc.scalar.activation(xT_sb, x_ps, AF.Relu, bias=cb_sb)

            # gi partials -> AllReduce
            for j in range(6):
                zp = psZ.tile([1, 512], f32, tag="z")
                nc.tensor.matmul(zp, xT_sb, wih_sb[:, j * 512:(j + 1) * 512],
                                 start=True, stop=True)
                gcp = sp.tile([1, 512], f32, tag="gcp")
                nc.vector.tensor_add(gcp, zp,
                                     bih8_sb[:, j * 512:(j + 1) * 512])
                dma(out=gi_in[0:1, j * 512:(j + 1) * 512], in_=gcp)
            allreduce(gi_in, gi_out)          # CC stream slot 3

            ggi = sp.tile([1, 3 * H], f32)
            dma(out=ggi, in_=gi_out[:])
            ggh = sp.tile([1, 3 * H], f32)
            dma(out=ggh, in_=gh_out[:])

            # gates (PyTorch order r,z,n); biases pre-folded into partials
            t1 = sp.tile([1, 2 * H], f32)
            nc.vector.tensor_add(t1, ggi[:, 0:2 * H], ggh[:, 0:2 * H])
            rz = sp.tile([1, 2 * H], f32)
            nc.scalar.activation(rz, t1, AF.Sigmoid)
            hnt = sp.tile([1, H], f32)
            nc.vector.tensor_mul(hnt, rz[:, 0:H], ggh[:, 2 * H:3 * H])
            npre = sp.tile([1, H], f32)
            nc.vector.tensor_add(npre, ggi[:, 2 * H:3 * H], hnt)
            nn = sp.tile([1, H], f32)
            nc.scalar.activation(nn, npre, AF.Tanh)
            dd = sp.tile([1, H], f32)
            nc.vector.tensor_sub(dd, hrow_sb, nn)
            nc.vector.tensor_mul(dd, rz[:, H:2 * H], dd)
            hnew = sp.tile([1, H], f32)
            nc.vector.tensor_add(hnew, nn, dd)
            dma(out=hn_t[:], in_=hnew)

            # h_new^T chunks [128, KT] (bf16) via K=1 matmuls with ones
            hnT_ps = psA.tile([128, KT], f32, tag="a")
            for k in range(KT):
                nc.tensor.matmul(hnT_ps[:, k:k + 1],
                                 hnew[:, k * 128:(k + 1) * 128], ones_sb,
                                 start=True, stop=True)
            hnT_sb = sp.tile([128, KT], bf16)
            nc.vector.tensor_copy(hnT_sb, hnT_ps)

            # ---- streamed output projection: z^T[p, vc] accumulation ----
            zT_ps = psT.tile([128, NVC], f32, tag="zt")
            vc0 = 0
            for nvc in VCBLK:
                owt = owp.tile([128, nvc * KT * 128], bf16, tag="ow")
                off = KT * 128 * vc0
                nc.scalar.dma_start(
                    out=owt, in_=ow_t[:, off:off + nvc * KT * 128])
                for v in range(nvc):
                    for k in range(KT):
                        nc.tensor.matmul(
                            zT_ps[:, vc0 + v:vc0 + v + 1],
                            owt[:, (v * KT + k) * 128:(v * KT + k + 1) * 128],
                            hnT_sb[:, k:k + 1],
                            start=(k == 0), stop=(k == KT - 1))
                vc0 += nvc

            # epilogue: z = z^T + obT; sumexp; AllReduce; out = z - ln(S)
            zb = sp.tile([128, NVC], f32)
            nc.vector.tensor_add(zb, zT_ps, obT_sb)
            ez = sp.tile([128, NVC], f32)
            acc = sp.tile([128, 1], f32)
            nc.scalar.activation(ez, zb, AF.Exp, accum_out=acc)
            se_ps = psA.tile([1, 1], f32, tag="a")
            nc.tensor.matmul(se_ps, acc, ones128, start=True, stop=True)
            s_loc = sp.tile([1, 1], f32)
            nc.vector.tensor_copy(s_loc, se_ps)
            dma(out=s_in[:], in_=s_loc)
            allreduce(s_in, s_out)            # CC stream slot 4
            sb128 = sp.tile([128, 1], f32)
            bcast = bass.AP(tensor=s_out, offset=0, ap=[[0, 128], [1, 1]])
            dma(out=sb128, in_=bcast)
            lnv = sp.tile([128, 1], f32)
            nc.scalar.activation(lnv, sb128, AF.Ln)
            nlse = sp.tile([128, 1], f32)
            nc.scalar.mul(nlse, lnv, -1.0)
            outT = sp.tile([128, NVC], f32)
            nc.scalar.activation(outT, zb, AF.Identity, bias=nlse)
            dma(out=zT_t[:], in_=outT)

    nc.compile()
    return nc


def shard_inputs(inputs):
    """Full inputs -> list of 8 per-core in_maps."""
    f = lambda x: np.ascontiguousarray(np.asarray(x), dtype=np.float32)
    idx = int(np.asarray(inputs["input_idx"]).reshape(-1)[0])
    h = f(inputs["hidden"]).reshape(1, H)
    enc = f(inputs["encoder_outputs"])                      # [L, ENC]
    emb_row = f(inputs["emb"][idx]).reshape(1, H)
    aw = f(inputs["attn_w_W"])                              # [3H, H]
    ab = f(inputs["attn_w_b"])                              # [H]
    av = f(inputs["attn_v_W"])                              # [H, 1]
    cw = f(inputs["comb_W"])
    cb = f(inputs["comb_b"])
    wih = f(inputs["gru_wih"])                              # [H, 3H]
    whh = f(inputs["gru_whh"])
    bih = f(inputs["gru_bih"])                              # [3H]
    bhh = f(inputs["gru_bhh"])
    ow = f(inputs["out_W"])                                 # [H, V]
    ob = f(inputs["out_b"])                                 # [V]

    ow_pad = np.full((H, VPAD), 0.0, dtype=np.float32)
    ow_pad[:, :V] = ow
    ob_pad = np.full((VPAD,), PAD_BIAS, dtype=np.float32)
    ob_pad[:V] = ob

    bih8 = (bih / 8.0).reshape(1, 3 * H).astype(np.float32)
    bhh8 = (bhh / 8.0).reshape(1, 3 * H).astype(np.float32)
    # [128, 16, 50]: [p, t, l] = enc[l, t*128+p]
    encTr = np.ascontiguousarray(enc.T.reshape(ENC // 128, 128, L)
                                 .transpose(1, 0, 2))
    hT = np.ascontiguousarray(h.reshape(KT, 128).T)          # [128, KT]
    embT = np.ascontiguousarray(emb_row.reshape(KT, 128).T)  # [128, KT]

    def colshard_kpm(w, s, dt=np.float32):
        # [3H, 128] col shard -> [128, 24, 128]: [p, t, m] = w[t*128+p, s][m]
        return np.ascontiguousarray(
            w[:, s].reshape(24, 128, 128).transpose(1, 0, 2).astype(dt))

    bf = ml_dtypes.bfloat16
    maps = []
    for c in range(NCORES):
        s = slice(c * 128, (c + 1) * 128)
        vs = slice(c * VS, (c + 1) * VS)
        # ow tiles: [p, (vc, k, c)] with [128h,128v] PE-stationary tiles
        ow4 = ow_pad[:, vs].reshape(KT, 128, NVC, 128)   # [k, p, vc, c]
        owr = np.ascontiguousarray(
            ow4.transpose(1, 2, 0, 3).reshape(128, KT * VS).astype(bf))
        obT = np.ascontiguousarray(
            ob_pad[vs].reshape(NVC, 128).T)              # [128, NVC]
        sm = np.concatenate([
            hT, embT,
            ab[s].reshape(128, 1), av[s, :].reshape(128, 1),
            cb[s].reshape(128, 1), h[0, s].reshape(128, 1),
        ], axis=1).astype(np.float32)
        maps.append({
            "h_full": h,
            "enc": enc,
            "encT": encTr,
            "aw": colshard_kpm(aw, s),
            "cw": colshard_kpm(cw, s),
            "smalls": np.ascontiguousarray(sm),
            "wih": np.ascontiguousarray(wih[s, :]),
            "whh": np.ascontiguousarray(whh[s, :]),
            "bih8": bih8,
            "bhh8": bhh8,
            "ow": owr,
            "obT": obT,
        })
    return maps


def assemble_outputs(results):
    # zT [128, NVC] per core: logprob[vc*128+p] = zT[p, vc]
    shards = [np.asarray(results[c]["zT_out"]).T.reshape(-1)
              for c in range(NCORES)]
    z = np.concatenate(shards)[:V].reshape(1, V)
    hn = np.asarray(results[0]["hn_out"]).reshape(1, 1, H).astype(np.float32)
    aw = np.asarray(results[0]["attw_out"]).astype(np.float32)
    return np.ascontiguousarray(z, dtype=np.float32), hn, aw


def kernel(**inputs):
    from concourse import bass_utils
    if "nc" not in _CACHE:
        _CACHE["nc"] = _build()
    nc = _CACHE["nc"]
    in_maps = shard_inputs(inputs)
    trace = bool(int(os.environ.get("KERNEL_TRACE", "0")))
    res = bass_utils.run_bass_kernel_spmd(
        nc, in_maps, core_ids=list(range(NCORES)), trace=trace)
    _CACHE["last_result"] = res
    return assemble_outputs(res.results)


# revision 17
# speedup vs baseline: 1.0689x; 1.0547x over previous
"""AttnDecoderRNN single-step decoder, tensor-parallel over 8 TRN2 NeuronCores.

Sharding (hardcoded for H=1024, V=50257, L=50, ENC=2048, 8 cores):
  - attn_w_W col-sharded fp32; attn_v_W row-sharded; comb_W col-sharded bf16
  - gru_wih / gru_whh row-sharded bf16 (K-sharded partials, AllReduced)
  - out_W col-sharded bf16 over vocab, padded 50257 -> 8*6400 = 51200,
    streamed as PE-stationary [128h,128v] tiles accumulating z^T [128,50]
  - embedding: host selects the one needed row (extreme vocab shard)
  - log_softmax via sharded sum-exp + scalar AllReduce (no max subtraction:
    logits for this model are O(3), exp is safe in fp32)
Collectives in stream order: e[50] AR, gh[3072] AR (attention-independent,
hidden behind the entry-barrier/e-AR), gi[3072] AR, sumexp[1] AR.
"""

import os
import numpy as np
import ml_dtypes

H = 1024
V = 50257
L = 50
ENC = 2048
NCORES = 8
KT = H // 128               # 8 h-chunks of 128
NVC = 50                    # vocab chunks of 128 per core
VS = NVC * 128              # 6400 per-core padded vocab shard (8*6400=51200)
VPAD = NCORES * VS
VCBLK = [4] * 12 + [2]          # vocab-chunk blocks for the ow stream
PAD_BIAS = -1.0e9           # pad-lane bias so exp() -> 0

_CACHE = {}


def _build():
    import concourse.bass as bass
    import concourse.mybir as mybir
    import concourse.tile as tile
    from concourse import bacc

    f32 = mybir.dt.float32
    f32r = mybir.dt.float32r
    bf16 = mybir.dt.bfloat16
    AF = mybir.ActivationFunctionType
    nc = bacc.Bacc("TRN2", target_bir_lowering=False, debug=False,
                   num_devices=NCORES)

    def din(name, shape, dt=f32):
        return nc.declare_dram_parameter(name, list(shape), dt, isOutput=False)

    def dout(name, shape):
        return nc.declare_dram_parameter(name, list(shape), f32, isOutput=True)

    # inputs (per-core shards, host-prearranged layouts)
    h_t = din("h_full", [1, H])
    enc_t = din("enc", [L, ENC])
    encT_t = din("encT", [128, ENC // 128, L])
    aw_t = din("aw", [128, 24, 128])
    cw_t = din("cw", [128, 24, 128])
    # merged small columns: [hT(8) | embT(8) | ab | av | cb | hc] = [128, 20]
    sm_t = din("smalls", [128, 2 * KT + 4])
    wih_t = din("wih", [128, 3 * H])
    whh_t = din("whh", [128, 3 * H])
    bih8_t = din("bih8", [1, 3 * H])
    bhh8_t = din("bhh8", [1, 3 * H])
    ow_t = din("ow", [128, KT * VS], bf16)   # [p, (vc, k, c)] tile-major
    obT_t = din("obT", [128, NVC])           # [p, vc] = ob[vc*128+p]
    # outputs
    zT_t = dout("zT_out", [128, NVC])        # [p, vc] = logprob[vc*128+p]
    hn_t = dout("hn_out", [1, H])
    atw_t = dout("attw_out", [1, L])
    # collective bounce buffers (internal DRAM)
    e_in = nc.dram_tensor("cc_e_in", [1, L], f32)
    e_out = nc.dram_tensor("cc_e_out", [1, L], f32)
    gh_in = nc.dram_tensor("cc_gh_in", [1, 3 * H], f32)
    gh_out = nc.dram_tensor("cc_gh_out", [1, 3 * H], f32)
    gi_in = nc.dram_tensor("cc_gi_in", [1, 3 * H], f32)
    gi_out = nc.dram_tensor("cc_gi_out", [1, 3 * H], f32)
    s_in = nc.dram_tensor("cc_s_in", [1, 1], f32)
    s_out = nc.dram_tensor("cc_s_out", [1, 1], f32)

    RG = [list(range(NCORES))]

    def allreduce(src, dst):
        nc.gpsimd.collective_compute(
            "AllReduce", mybir.AluOpType.add, replica_groups=RG,
            ins=[src[:].opt()], outs=[dst[:].opt()])

    with tile.TileContext(nc) as tc:
        with (
            tc.tile_pool(name="wpool", bufs=1) as wp,
            tc.tile_pool(name="bigw", bufs=2) as bw,
            tc.tile_pool(name="spool", bufs=1) as sp,
            tc.tile_pool(name="owpool", bufs=6) as owp,
            tc.tile_pool(name="psA", bufs=3, space="PSUM") as psA,
            tc.tile_pool(name="psZ", bufs=3, space="PSUM") as psZ,
            tc.tile_pool(name="psT", bufs=1, space="PSUM") as psT,
        ):
            dma = nc.sync.dma_start

            # ---- persistent loads, critical-chain first -----------------
            sm_sb = wp.tile([128, 2 * KT + 4], f32)
            dma(out=sm_sb, in_=sm_t[:])
            hT_sb = sm_sb[:, 0:KT]
            embT_sb = sm_sb[:, KT:2 * KT]
            ab_sb = sm_sb[:, 2 * KT:2 * KT + 1]
            av_sb = sm_sb[:, 2 * KT + 1:2 * KT + 2]
            cb_sb = sm_sb[:, 2 * KT + 2:2 * KT + 3]
            hc_sb = sm_sb[:, 2 * KT + 3:2 * KT + 4]
            encT_sb = wp.tile([128, ENC // 128, L], f32)
            dma(out=encT_sb, in_=encT_t[:])
            enc_sb = wp.tile([L, ENC], f32)
            dma(out=enc_sb, in_=enc_t[:])
            aw_sb = bw.tile([128, 24, 128], f32, tag="bigw")
            dma(out=aw_sb, in_=aw_t[:])
            whh_sb = bw.tile([128, 3 * H], f32, tag="bigw")
            nc.scalar.dma_start(out=whh_sb, in_=whh_t[:])
            cw_sb = bw.tile([128, 24, 128], f32, tag="bigw")
            nc.scalar.dma_start(out=cw_sb, in_=cw_t[:])
            wih_sb = bw.tile([128, 3 * H], f32, tag="bigw")
            nc.scalar.dma_start(out=wih_sb, in_=wih_t[:])
            bih8_sb = wp.tile([1, 3 * H], f32)
            nc.scalar.dma_start(out=bih8_sb, in_=bih8_t[:])
            bhh8_sb = wp.tile([1, 3 * H], f32)
            nc.scalar.dma_start(out=bhh8_sb, in_=bhh8_t[:])
            hrow_sb = wp.tile([1, H], f32)
            nc.scalar.dma_start(out=hrow_sb, in_=h_t[:])
            obT_sb = wp.tile([128, NVC], f32)
            nc.scalar.dma_start(out=obT_sb, in_=obT_t[:])

            ones_sb = wp.tile([1, 1], f32)
            nc.vector.memset(ones_sb, 1.0)
            ones128 = wp.tile([128, 1], f32)
            nc.vector.memset(ones128, 1.0)

            # ---- attention: e_partial = relu(att_cat @ aw + ab) @ av ----
            v1_ps = psA.tile([128, 1], f32, tag="a")
            for k in range(KT):
                nc.tensor.matmul(v1_ps, aw_sb[:, k, :], hT_sb[:, k:k + 1],
                                 start=(k == 0), stop=(k == KT - 1))
            v1_sb = sp.tile([128, 1], f32)
            nc.vector.tensor_copy(v1_sb, v1_ps)
            bias2 = sp.tile([128, 1], f32)
            nc.vector.tensor_add(bias2, v1_sb, ab_sb)
            ep = psA.tile([128, L], f32, tag="a")
            for k in range(16):
                nc.tensor.matmul(ep, aw_sb[:, KT + k, :], encT_sb[:, k, :],
                                 start=(k == 0), stop=(k == 15))
            r_sb = sp.tile([128, L], f32)
            nc.scalar.activation(r_sb, ep, AF.Relu, bias=bias2)
            e_ps = psA.tile([1, L], f32, tag="a")
            nc.tensor.matmul(e_ps, av_sb, r_sb, start=True, stop=True)
            e_sb = sp.tile([1, L], f32)
            nc.vector.tensor_copy(e_sb, e_ps)
            dma(out=e_in[:], in_=e_sb)
            allreduce(e_in, e_out)            # CC stream slot 1

            # gh partials (attention-independent) ride behind the e-AR
            for j in range(6):
                zp = psZ.tile([1, 512], f32, tag="z")
                nc.tensor.matmul(zp, hc_sb, whh_sb[:, j * 512:(j + 1) * 512],
                                 start=True, stop=True)
                gcp = sp.tile([1, 512], f32, tag="gcp")
                nc.vector.tensor_add(gcp, zp,
                                     bhh8_sb[:, j * 512:(j + 1) * 512])
                dma(out=gh_in[0:1, j * 512:(j + 1) * 512], in_=gcp)
            allreduce(gh_in, gh_out)          # CC stream slot 2

            # x^T emb-part (attention-independent), fills the e-AR wait
            x_ps = psA.tile([128, 1], f32, tag="a")
            for k in range(KT):
                nc.tensor.matmul(x_ps, cw_sb[:, k, :], embT_sb[:, k:k + 1],
                                 start=(k == 0), stop=False)

            e_row = sp.tile([1, L], f32)
            dma(out=e_row, in_=e_out[:])

            # softmax over L (with max subtraction; all on partition 0)
            m1 = sp.tile([1, 1], f32)
            nc.vector.reduce_max(m1, e_row, axis=mybir.AxisListType.X)
            negm = sp.tile([1, 1], f32)
            nc.scalar.mul(negm, m1, -1.0)
            ew = sp.tile([1, L], f32)
            s_att = sp.tile([1, 1], f32)
            nc.scalar.activation(ew, e_row, AF.Exp, bias=negm,
                                 accum_out=s_att)
            inv = sp.tile([1, 1], f32)
            nc.vector.reciprocal(inv, s_att)
            attw_sb = sp.tile([1, L], f32)
            nc.scalar.mul(attw_sb, ew, inv)
            dma(out=atw_t[:], in_=attw_sb)

            # attn_w^T [L,1] via K=1 matmul with ones
            awT_ps = psA.tile([L, 1], f32, tag="a")
            nc.tensor.matmul(awT_ps, attw_sb, ones_sb, start=True, stop=True)
            awT_sb = sp.tile([L, 1], f32)
            nc.vector.tensor_copy(awT_sb, awT_ps)

            # attn_applied^T chunks: aaT[:, j] = enc[:, j*128:+128]^T @ attn_w^T
            aa_ps = psA.tile([128, ENC // 128], f32, tag="a")
            for j in range(ENC // 128):
                nc.tensor.matmul(aa_ps[:, j:j + 1],
                                 enc_sb[:, j * 128:(j + 1) * 128], awT_sb,
                                 start=True, stop=True)
            aaT_sb = sp.tile([128, ENC // 128], f32)
            nc.vector.tensor_copy(aaT_sb, aa_ps)

            # x^T attention part completes the accumulation
            for k in range(KT, 24):
                nc.tensor.matmul(x_ps, cw_sb[:, k, :],
                                 aaT_sb[:, k - KT:k - KT + 1],
                                 start=False, stop=(k == 23))
            xT_sb = sp.tile([128, 1], f32)
            nc.scalar.activation(xT_sb, x_ps, AF.Relu, bias=cb_sb)

            # gi partials -> AllReduce
            for j in range(6):
                zp = psZ.tile([1, 512], f32, tag="z")
                nc.tensor.matmul(zp, xT_sb, wih_sb[:, j * 512:(j + 1) * 512],
                                 start=True, stop=True)
                gcp = sp.tile([1, 512], f32, tag="gcp")
                nc.vector.tensor_add(gcp, zp,
                                     bih8_sb[:, j * 512:(j + 1) * 512])
                dma(out=gi_in[0:1, j * 512:(j + 1) * 512], in_=gcp)
            allreduce(gi_in, gi_out)          # CC stream slot 3

            ggi = sp.tile([1, 3 * H], f32)
            dma(out=ggi, in_=gi_out[:])
            ggh = sp.tile([1, 3 * H], f32)
            dma(out=ggh, in_=gh_out[:])

            # gates (PyTorch order r,z,n); biases pre-folded into partials
            t1 = sp.tile([1, 2 * H], f32)
            nc.vector.tensor_add(t1, ggi[:, 0:2 * H], ggh[:, 0:2 * H])
            rz = sp.tile([1, 2 * H], f32)
            nc.scalar.activation(rz, t1, AF.Sigmoid)
            hnt = sp.tile([1, H], f32)
            nc.vector.tensor_mul(hnt, rz[:, 0:H], ggh[:, 2 * H:3 * H])
            npre = sp.tile([1, H], f32)
            nc.vector.tensor_add(npre, ggi[:, 2 * H:3 * H], hnt)
            nn = sp.tile([1, H], f32)
            nc.scalar.activation(nn, npre, AF.Tanh)
            dd = sp.tile([1, H], f32)
            nc.vector.tensor_sub(dd, hrow_sb, nn)
            nc.vector.tensor_mul(dd, rz[:, H:2 * H], dd)
            hnew = sp.tile([1, H], f32)
            nc.vector.tensor_add(hnew, nn, dd)
            dma(out=hn_t[:], in_=hnew)

            # h_new^T chunks [128, KT] (bf16) via K=1 matmuls with ones
            hnT_ps = psA.tile([128, KT], f32, tag="a")
            for k in range(KT):
                nc.tensor.matmul(hnT_ps[:, k:k + 1],
                                 hnew[:, k * 128:(k + 1) * 128], ones_sb,
                                 start=True, stop=True)
            hnT_sb = sp.tile([128, KT], bf16)
            nc.vector.tensor_copy(hnT_sb, hnT_ps)

            # ---- streamed output projection: z^T[p, vc] accumulation ----
            zT_ps = psT.tile([128, NVC], f32, tag="zt")
            vc0 = 0
            for nvc in VCBLK:
                owt = owp.tile([128, nvc * KT * 128], bf16, tag="ow")
                off = KT * 128 * vc0
                nc.scalar.dma_start(
                    out=owt, in_=ow_t[:, off:off + nvc * KT * 128])
                for v in range(nvc):
                    for k in range(KT):
                        nc.tensor.matmul(
                            zT_ps[:, vc0 + v:vc0 + v + 1],
                            owt[:, (v * KT + k) * 128:(v * KT + k + 1) * 128],
                            hnT_sb[:, k:k + 1],
                            start=(k == 0), stop=(k == KT - 1))
                vc0 += nvc

            # epilogue: z = z^T + obT; sumexp; AllReduce; out = z - ln(S)
            zb = sp.tile([128, NVC], f32)
            nc.vector.tensor_add(zb, zT_ps, obT_sb)
            ez = sp.tile([128, NVC], f32)
            acc = sp.tile([128, 1], f32)
            nc.scalar.activation(ez, zb, AF.Exp, accum_out=acc)
            se_ps = psA.tile([1, 1], f32, tag="a")
            nc.tensor.matmul(se_ps, acc, ones128, start=True, stop=True)
            s_loc = sp.tile([1, 1], f32)
            nc.vector.tensor_copy(s_loc, se_ps)
            dma(out=s_in[:], in_=s_loc)
            allreduce(s_in, s_out)            # CC stream slot 4
            sb128 = sp.tile([128, 1], f32)
            bcast = bass.AP(tensor=s_out, offset=0, ap=[[0, 128], [1, 1]])
            dma(out=sb128, in_=bcast)
            lnv = sp.tile([128, 1], f32)
            nc.scalar.activation(lnv, sb128, AF.Ln)
            nlse = sp.tile([128, 1], f32)
            nc.scalar.mul(nlse, lnv, -1.0)
            outT = sp.tile([128, NVC], f32)
            nc.scalar.activation(outT, zb, AF.Identity, bias=nlse)
            dma(out=zT_t[:], in_=outT)

    nc.compile()
    return nc


def shard_inputs(inputs):
    """Full inputs -> list of 8 per-core in_maps."""
    f = lambda x: np.ascontiguousarray(np.asarray(x), dtype=np.float32)
    idx = int(np.asarray(inputs["input_idx"]).reshape(-1)[0])
    h = f(inputs["hidden"]).reshape(1, H)
    enc = f(inputs["encoder_outputs"])                      # [L, ENC]
    emb_row = f(inputs["emb"][idx]).reshape(1, H)
    aw = f(inputs["attn_w_W"])                              # [3H, H]
    ab = f(inputs["attn_w_b"])                              # [H]
    av = f(inputs["attn_v_W"])                              # [H, 1]
    cw = f(inputs["comb_W"])
    cb = f(inputs["comb_b"])
    wih = f(inputs["gru_wih"])                              # [H, 3H]
    whh = f(inputs["gru_whh"])
    bih = f(inputs["gru_bih"])                              # [3H]
    bhh = f(inputs["gru_bhh"])
    ow = f(inputs["out_W"])                                 # [H, V]
    ob = f(inputs["out_b"])                                 # [V]

    ow_pad = np.full((H, VPAD), 0.0, dtype=np.float32)
    ow_pad[:, :V] = ow
    ob_pad = np.full((VPAD,), PAD_BIAS, dtype=np.float32)
    ob_pad[:V] = ob

    bih8 = (bih / 8.0).reshape(1, 3 * H).astype(np.float32)
    bhh8 = (bhh / 8.0).reshape(1, 3 * H).astype(np.float32)
    # [128, 16, 50]: [p, t, l] = enc[l, t*128+p]
    encTr = np.ascontiguousarray(enc.T.reshape(ENC // 128, 128, L)
                                 .transpose(1, 0, 2))
    hT = np.ascontiguousarray(h.reshape(KT, 128).T)          # [128, KT]
    embT = np.ascontiguousarray(emb_row.reshape(KT, 128).T)  # [128, KT]

    def colshard_kpm(w, s, dt=np.float32):
        # [3H, 128] col shard -> [128, 24, 128]: [p, t, m] = w[t*128+p, s][m]
        return np.ascontiguousarray(
            w[:, s].reshape(24, 128, 128).transpose(1, 0, 2).astype(dt))

    bf = ml_dtypes.bfloat16
    maps = []
    for c in range(NCORES):
        s = slice(c * 128, (c + 1) * 128)
        vs = slice(c * VS, (c + 1) * VS)
        # ow tiles: [p, (vc, k, c)] with [128h,128v] PE-stationary tiles
        ow4 = ow_pad[:, vs].reshape(KT, 128, NVC, 128)   # [k, p, vc, c]
        owr = np.ascontiguousarray(
            ow4.transpose(1, 2, 0, 3).reshape(128, KT * VS).astype(bf))
        obT = np.ascontiguousarray(
            ob_pad[vs].reshape(NVC, 128).T)              # [128, NVC]
        sm = np.concatenate([
            hT, embT,
            ab[s].reshape(128, 1), av[s, :].reshape(128, 1),
            cb[s].reshape(128, 1), h[0, s].reshape(128, 1),
        ], axis=1).astype(np.float32)
        maps.append({
            "h_full": h,
            "enc": enc,
            "encT": encTr,
            "aw": colshard_kpm(aw, s),
            "cw": colshard_kpm(cw, s),
            "smalls": np.ascontiguousarray(sm),
            "wih": np.ascontiguousarray(wih[s, :]),
            "whh": np.ascontiguousarray(whh[s, :]),
            "bih8": bih8,
            "bhh8": bhh8,
            "ow": owr,
            "obT": obT,
        })
    return maps


def assemble_outputs(results):
    # zT [128, NVC] per core: logprob[vc*128+p] = zT[p, vc]
    shards = [np.asarray(results[c]["zT_out"]).T.reshape(-1)
              for c in range(NCORES)]
    z = np.concatenate(shards)[:V].reshape(1, V)
    hn = np.asarray(results[0]["hn_out"]).reshape(1, 1, H).astype(np.float32)
    aw = np.asarray(results[0]["attw_out"]).astype(np.float32)
    return np.ascontiguousarray(z, dtype=np.float32), hn, aw


def kernel(**inputs):
    from concourse import bass_utils
    if "nc" not in _CACHE:
        _CACHE["nc"] = _build()
    nc = _CACHE["nc"]
    in_maps = shard_inputs(inputs)
    trace = bool(int(os.environ.get("KERNEL_TRACE", "0")))
    res = bass_utils.run_bass_kernel_spmd(
        nc, in_maps, core_ids=list(range(NCORES)), trace=trace)
    _CACHE["last_result"] = res
    return assemble_outputs(res.results)
